# revision 26
# baseline (speedup 1.0000x reference)
"""
Self-contained Bass/Trainium2 kernel for the 2-layer 2-head GAT
(nn_GATNet): kernel(**inputs) takes the FULL unsharded inputs
(x [50000,128] f32, edge_index [2,800000] int64, W0, attn0, W1, attn1)
and returns the FULL [50000, 1] f32 output, computed on 8 TRN2
NeuronCores via bass_utils.run_bass_kernel_spmd.

Strategy
--------
Destination-node sharding: core r owns destination nodes [r*S, (r+1)*S).
All edges pointing into that slice are processed by that core, so the
segment softmax and the weighted scatter are purely core-local; the only
collectives are AllGathers of the (node-major) gather tables.

Per layer, node records live in a DRAM table in *table position* order
(row = core*S_pad + rank, ranks snake-sorted by (deg_hi, deg_lo) so
padded-CSR batches are uniform):
  L0: htab [2*HALFT, 256] fp16  (row = [h0(128), a_c0(2), a_r0(2), pad])
  L1: t1tab [2*HALFT, 128] fp16 (row = [h1(2), a_c1(2), a_r1(2), pad])
Row S of each core's slice is a DUMMY record (h=0, a_c=-30000): padding
slots gather it and get w = exp(lrelu(-30000+a_r)) == 0 exactly, so no
separate additive mask is needed.

Edge phase runs in GROUPS of nb consecutive batches sharing uniform
slot counts (klo, khi).  Per group (one gather pair + ~18 fused ops):
  g = dma_gather(lo) | dma_gather(hi)          # [128, nb*(klo+khi), rec]
  t = a_c(g) + a_r(dest)                       # f32 [128, used, H]
  w = exp(max(t, 0.2t))                        # fp16, no max-subtract
  s = sum_k w ; acc = sum_k w * feat(g)
  x1/out = 0.5 * (acc_h0/max(s0,eps) + acc_h1/max(s1,eps))

int16 gather indices only reach 32767, so lo (cores 0-3) and hi
(cores 4-7) halves of each table are gathered separately.
"""
import sys
if "/opt/trn_rl_repo" not in sys.path:
    sys.path.insert(0, "/opt/trn_rl_repo")

from contextlib import ExitStack
from dataclasses import dataclass, field

import numpy as np

import concourse.bass as bass
import concourse.bacc as bacc
import concourse.mybir as mybir
import concourse.tile as tile
from concourse import masks

F32 = mybir.dt.float32
F16 = mybir.dt.float16
I16 = mybir.dt.int16

IN_CH = 128
HID = 64
HEADS = 2
OUT_CH = 1
REC0 = 256                  # fp16 gather record L0: [h(128), a_c(2), pad] 512B
REC1 = 128                  # fp16 gather record L1: [h1(2), a_c(2), a_r(2)] 256B
NEGC = -1000.0              # dummy-record a_c: w = exp(0.2*(NEGC+a_r)) == 0
CS0 = 48                    # slot budget per L0 group (g tile [128, CS0, 256] f16)
CS1 = 48                    # slot budget per L1 group


# ----------------------------------------------------------------------------
# Host-side preprocessing
# ----------------------------------------------------------------------------

@dataclass
class Schedule:
    N: int
    NC: int
    S: int                      # nodes per core
    B: int                      # batches per core
    S_pad: int                  # B*128
    HALFT: int                  # 4*S_pad (rows per table half)
    KL: list = field(default_factory=list)     # per-batch lo slot count
    KH: list = field(default_factory=list)
    groups0: list = field(default_factory=list)
    groups1: list = field(default_factory=list)
    W: int = 0                  # idx tile free dim (both layers)


def _snake_perm(dlo, dhi):
    """Sort by (deg_hi, deg_lo), alternating direction per deg_hi group."""
    perm = np.argsort(dhi * 100000 + dlo, kind="stable")
    vals = dhi[perm]
    out = []
    i = 0
    g = 0
    while i < len(perm):
        j = i
        while j < len(perm) and vals[j] == vals[i]:
            j += 1
        grp = perm[i:j]
        if g % 2 == 1:
            grp = grp[::-1]
        out.append(grp)
        i = j
        g += 1
    return np.concatenate(out)


def _build_groups(KL, KH, cap, B):
    groups = []
    b = 0
    while b < B:
        nb = 1
        klo, khi = KL[b], KH[b]
        while b + nb < B:
            klo2 = max(klo, KL[b + nb])
            khi2 = max(khi, KH[b + nb])
            if (nb + 1) * (klo2 + khi2) > cap:
                break
            klo, khi = klo2, khi2
            nb += 1
        groups.append(dict(b0=b, nb=nb, klo=int(klo), khi=int(khi)))
        b += nb
    return groups


def build_host_data(x, edge_index, W0, attn0, W1, attn1, NC=8):
    """Returns (schedule, per_core_inputs, unperm); unperm[g] = original
    node id at output position g (= core*S + rank)."""
    x = np.asarray(x, np.float32)
    edge_index = np.asarray(edge_index)
    W0 = np.asarray(W0, np.float32)
    attn0 = np.asarray(attn0, np.float32)
    W1 = np.asarray(W1, np.float32)
    attn1 = np.asarray(attn1, np.float32)

    N = x.shape[0]
    assert N % (2 * NC) == 0, (N, NC)
    S = N // NC
    B = (S + 127) // 128
    S_pad = B * 128
    HALFT = (NC // 2) * S_pad
    half = N // 2
    DUMMY = S  # relative dummy row within core 0 (lo) / core NC/2 (hi) slice

    row = edge_index[0].astype(np.int64)
    col = edge_index[1].astype(np.int64)

    sched = Schedule(N=N, NC=NC, S=S, B=B, S_pad=S_pad, HALFT=HALFT)

    # ---- per-core permutation + degree tables ----
    perms = []
    rank = np.empty(N, np.int64)        # rank of node within its core
    core_edges = []
    KLm = np.zeros((NC, B), np.int64)
    KHm = np.zeros((NC, B), np.int64)
    for r in range(NC):
        lo_n, hi_n = r * S, (r + 1) * S
        m = (row >= lo_n) & (row < hi_n)
        er, ec = row[m] - lo_n, col[m]
        elo = ec < half
        deg_lo = np.bincount(er[elo], minlength=S)
        deg_hi = np.bincount(er[~elo], minlength=S)
        perm = _snake_perm(deg_lo, deg_hi)
        rank_of = np.empty(S, np.int64)
        rank_of[perm] = np.arange(S)
        perms.append(perm)
        rank[lo_n:hi_n] = rank_of
        core_edges.append((rank_of[er], ec, elo))
        dl = np.pad(deg_lo[perm], (0, S_pad - S))
        dh = np.pad(deg_hi[perm], (0, S_pad - S))
        KLm[r] = dl.reshape(B, 128).max(1)
        KHm[r] = dh.reshape(B, 128).max(1)

    unperm = np.empty(N, np.int64)
    pos_out = (np.arange(N) // S) * S + rank  # output position per node
    unperm[pos_out] = np.arange(N)

    sched.KL = KLm.max(0).tolist()
    sched.KH = KHm.max(0).tolist()
    sched.groups0 = _build_groups(sched.KL, sched.KH, CS0, B)
    sched.groups1 = _build_groups(sched.KL, sched.KH, CS1, B)

    # idx column offsets (8 cols per 128 idxs)
    off = 0
    for groups in (sched.groups0, sched.groups1):
        for gr in groups:
            gr["off_lo"] = off
            off += 8 * gr["nb"] * gr["klo"]
            gr["off_hi"] = off
            off += 8 * gr["nb"] * gr["khi"]
    sched.W = max(off, 16)

    # table row (relative to its half) per source node
    core_of = np.arange(N) // S
    tab_rel = (core_of % (NC // 2)) * S_pad + rank  # 0..HALFT-1

    # ---- per-core idx tiles + permuted xT slices ----
    per_core = []
    for r in range(NC):
        er, ec, elo = core_edges[r]
        lists_lo = [[] for _ in range(S_pad)]
        lists_hi = [[] for _ in range(S_pad)]
        for q, c, lo in zip(er, ec, elo):
            (lists_lo if lo else lists_hi)[q].append(tab_rel[c])

        idx = np.zeros((16, sched.W), np.int16)
        for groups in (sched.groups0, sched.groups1):
            for gr in groups:
                b0, nb = gr["b0"], gr["nb"]
                for kk, offs, lists in ((gr["klo"], gr["off_lo"], lists_lo),
                                        (gr["khi"], gr["off_hi"], lists_hi)):
                    if kk == 0:
                        continue
                    blk = np.full(nb * kk * 128, DUMMY, np.int64)
                    for n in range(nb):
                        base = (b0 + n) * 128
                        for p in range(128):
                            lst = lists[base + p]
                            for k, src in enumerate(lst):
                                blk[(n * kk + k) * 128 + p] = src
                    assert blk.max() <= 32767
                    idx[:, offs:offs + 8 * nb * kk] = \
                        blk.reshape(8 * nb * kk, 16).T
        idx_tile = np.broadcast_to(
            idx[None, :, :], (8, 16, sched.W)).reshape(128, sched.W).copy()

        xpT = np.zeros((IN_CH, S_pad), np.float16)
        xpT[:, :S] = x[r * S + perms[r]].T.astype(np.float16)
        per_core.append({"xpT": xpT, "idx": np.ascontiguousarray(idx_tile)})

    # replicated full-table x (table-position order), shared by all cores
    xT = np.zeros((IN_CH, 2 * HALFT), np.float16)
    for r in range(NC):
        xT[:, r * S_pad:r * S_pad + S] = per_core[r]["xpT"][:, :S]
    for d in per_core:
        d["xT"] = xT

    # ---- weights / constants (shared across cores) ----
    wcat0 = np.zeros((IN_CH, REC0), np.float32)
    wcat0[:, :128] = W0
    for h in range(HEADS):
        blk = W0[:, h * HID:(h + 1) * HID].astype(np.float64)
        wcat0[:, 128 + h] = (blk @ attn0[h, HID:].astype(np.float64)).astype(np.float32)
        wcat0[:, 130 + h] = (blk @ attn0[h, :HID].astype(np.float64)).astype(np.float32)
    wcat0 = wcat0.astype(np.float16)

    wcat1 = np.zeros((HID, REC1), np.float32)
    wcat1[:, 0:2] = W1
    for h in range(HEADS):
        wcat1[:, 2 + h] = W1[:, h] * attn1[h, 1]
        wcat1[:, 4 + h] = W1[:, h] * attn1[h, 0]
    wcat1 = wcat1.astype(np.float16)

    for d in per_core:
        d.update({"wcat0": wcat0, "wcat1": wcat1})
    return sched, per_core, unperm


# ----------------------------------------------------------------------------
# Numpy emulation of the device algorithm (for fast validation)
# ----------------------------------------------------------------------------

def emulate(sched, per_core, unperm):
    NC, B, S, S_pad, HALFT = (sched.NC, sched.B, sched.S, sched.S_pad,
                              sched.HALFT)
    f16 = lambda a: a.astype(np.float16).astype(np.float32)

    def lrelu_exp(t):
        return np.exp(np.maximum(t, 0.2 * t), dtype=np.float32)

    # ---- layer-0 node phase ----
    htab = np.zeros((2 * HALFT, REC0), np.float32)
    A0 = np.zeros((NC, S_pad, HEADS), np.float32)
    for r in range(NC):
        d = per_core[r]
        hm = f16(d["xpT"].astype(np.float32).T @ d["wcat0"].astype(np.float32))
        hm[S, 128:130] = NEGC
        htab[r * S_pad:(r + 1) * S_pad] = hm
        A0[r] = hm[:, 130:132]

    def edge_phase(r, groups, tab, rec, A, nfeat):
        d = per_core[r]
        idx = d["idx"][:16]
        res = np.zeros((S_pad, HEADS, nfeat), np.float32)
        ss = np.zeros((S_pad, HEADS), np.float32)
        for gr in groups:
            b0, nb, klo, khi = gr["b0"], gr["nb"], gr["klo"], gr["khi"]
            used = nb * (klo + khi)
            if used == 0:
                continue
            G = np.zeros((128, used, rec), np.float32)
            for kk, offs, base, c0 in ((klo, gr["off_lo"], 0, 0),
                                       (khi, gr["off_hi"], HALFT, nb * klo)):
                for j in range(nb * kk * 128):
                    jj = idx[(j % 16), offs + j // 16]
                    G[j % 128, c0 + j // 128] = tab[base + int(jj)]
            # t = a_c + a_r ; per-batch a_r
            ac = G[:, :, 128:130] if rec == REC0 else G[:, :, 2:4]
            t = np.zeros((128, used, HEADS), np.float32)
            for n in range(nb):
                a_r = A[r, (b0 + n) * 128:(b0 + n + 1) * 128]      # [128, H]
                t[:, n * klo:(n + 1) * klo] = ac[:, n * klo:(n + 1) * klo] \
                    + a_r[:, None, :]
                o = nb * klo + n * khi
                t[:, o:o + khi] = ac[:, o:o + khi] + a_r[:, None, :]
            w = f16(lrelu_exp(t))
            feat = (G[:, :, :128].reshape(128, used, HEADS, HID)
                    if rec == REC0 else G[:, :, 0:2][..., None])
            pt = f16(feat * w[..., None])
            for n in range(nb):
                sl_lo = slice(n * klo, (n + 1) * klo)
                sl_hi = slice(nb * klo + n * khi, nb * klo + (n + 1) * khi)
                rows = slice((b0 + n) * 128, (b0 + n + 1) * 128)
                ss[rows] = w[:, sl_lo].sum(1) + w[:, sl_hi].sum(1)
                res[rows] = pt[:, sl_lo].sum(1) + pt[:, sl_hi].sum(1)
        ss = np.maximum(ss, 1e-30)
        return 0.5 * (res[:, 0] / ss[:, :1] + res[:, 1] / ss[:, 1:])

    # ---- layer-0 edge phase -> x1 ----
    x1 = np.zeros((NC, S_pad, HID), np.float32)
    for r in range(NC):
        x1[r] = f16(np.maximum(
            edge_phase(r, sched.groups0, htab, REC0, A0, HID), 0.0))

    # ---- layer-1 node phase ----
    t1tab = np.zeros((2 * HALFT, REC1), np.float32)
    A1 = np.zeros((NC, S_pad, HEADS), np.float32)
    for r in range(NC):
        t1 = f16(x1[r] @ per_core[r]["wcat1"].astype(np.float32))
        t1[S, 2:4] = NEGC
        t1tab[r * S_pad:(r + 1) * S_pad] = t1
        A1[r] = t1[:, 4:6]

    # ---- layer-1 edge phase -> out ----
    out = np.zeros((NC, S_pad), np.float32)
    for r in range(NC):
        out[r] = edge_phase(r, sched.groups1, t1tab, REC1, A1, 1)[:, 0]

    full = out[:, :S].reshape(-1)
    res = np.empty((sched.N, 1), np.float32)
    res[unperm] = full[:, None]
    return res


# ----------------------------------------------------------------------------
# Device kernel builder
# ----------------------------------------------------------------------------

def build_kernel(sched: Schedule, gbufs=4):
    NC, B, S, S_pad, HALFT, W = (sched.NC, sched.B, sched.S, sched.S_pad,
                                 sched.HALFT, sched.W)
    DPART = S % 128            # partition of the dummy row in the last batch
    nc = bacc.Bacc("TRN2", target_bir_lowering=False, debug=False,
                   num_devices=NC, num_swdge_queues=4)

    xpT_d = nc.dram_tensor("xpT", [IN_CH, S_pad], F16, kind="ExternalInput")
    xT_d = nc.dram_tensor("xT", [IN_CH, 2 * HALFT], F16, kind="ExternalInput")
    idx_d = nc.dram_tensor("idx", [128, W], I16, kind="ExternalInput")
    wcat0_d = nc.dram_tensor("wcat0", [IN_CH, REC0], F16, kind="ExternalInput")
    wcat1_d = nc.dram_tensor("wcat1", [HID, REC1], F16, kind="ExternalInput")
    out_d = nc.dram_tensor("out", [128, B], F32, kind="ExternalOutput")

    rg = [list(range(NC))]

    with tile.TileContext(nc) as tc, ExitStack() as ctx:
        aspace = "Shared" if NC > 4 else "Local"
        dram = ctx.enter_context(tc.tile_pool(name="dram", bufs=1, space="DRAM"))
        htab = dram.tile([2 * HALFT, REC0], F16)
        t1slice = dram.tile([S_pad, REC1], F16)
        t1tab = dram.tile([2 * HALFT, REC1], F16, addr_space=aspace)

        const = ctx.enter_context(tc.tile_pool(name="const", bufs=1))
        wcat0 = const.tile([IN_CH, REC0], F16)
        wcat1 = const.tile([HID, REC1], F16)
        ident = const.tile([128, 128], F16)
        xpT = const.tile([128, S_pad], F16)
        idx_sb = const.tile([128, W], I16)
        a0_sb = const.tile([128, B, HEADS], F32)
        a1_sb = const.tile([128, B, HEADS], F32)
        out_sb = const.tile([128, B], F32)
        x1_all = const.tile([128, B, HID], F16)

        nc.sync.dma_start(wcat0[:, :], wcat0_d[:, :])
        nc.sync.dma_start(wcat1[:, :], wcat1_d[:, :])
        nc.sync.dma_start(xpT[:, :], xpT_d[:, :])
        nc.sync.dma_start(idx_sb[:, :], idx_d[:, :])
        masks.make_identity(nc, ident[:, :])

        stage = ctx.enter_context(tc.tile_pool(name="stage", bufs=3))
        xin = ctx.enter_context(tc.tile_pool(name="xin", bufs=3))
        psum = ctx.enter_context(tc.tile_pool(name="psum", bufs=2, space="PSUM"))

        # ---------------- phase 1: replicated H table build ------------------
        # Every core computes the FULL table from the replicated xT input (no
        # AllGather, no collective barrier; lo-half gathers can start once the
        # lo half of htab is written). a_r for the core's OWN dest slice comes
        # from a tiny per-batch matmul against the per-core xpT input.
        for b in range(B):
            ps_ar = psum.tile([128, HEADS], F32, tag="ar")
            nc.tensor.matmul(ps_ar[:, :], xpT[:, b * 128:(b + 1) * 128],
                             wcat0[:, 130:132], start=True, stop=True)
            nc.vector.tensor_copy(a0_sb[:, b, :], ps_ar[:, :])

        TT = 2 * HALFT // 128
        XB = 4                       # table tiles per staged write
        for t0 in range(0, TT, XB):
            xt = xin.tile([128, XB * 128], F16, tag="x")
            nc.sync.dma_start(xt[:, :], xT_d[:, t0 * 128:(t0 + XB) * 128])
            ht = stage.tile([128, XB * REC0], F16, tag="ht")
            for j in range(XB):
                t = t0 + j
                ps_mm = psum.tile([128, REC0], F32, tag="mm")
                nc.tensor.matmul(ps_mm[:, :], xt[:, j * 128:(j + 1) * 128],
                                 wcat0[:, :], start=True, stop=True)
                if j % 2 == 0:
                    nc.scalar.activation(ht[:, j * REC0:(j + 1) * REC0],
                                         ps_mm[:, :],
                                         mybir.ActivationFunctionType.Copy)
                else:
                    nc.vector.tensor_copy(ht[:, j * REC0:(j + 1) * REC0],
                                          ps_mm[:, :])
                if t % B == B - 1:
                    # dummy row of this core-slice: a_c += NEGC*onehot(DPART)
                    nc.vector.scalar_tensor_tensor(
                        ht[:, j * REC0 + 128:j * REC0 + 130],
                        ident[:, DPART:DPART + 1].broadcast_to([128, HEADS]),
                        NEGC, ht[:, j * REC0 + 128:j * REC0 + 130],
                        op0=mybir.AluOpType.mult, op1=mybir.AluOpType.add)
            nc.sync.dma_start(
                bass.AP(htab.tensor, htab.offset + t0 * 128 * REC0,
                        [[REC0, 128], [128 * REC0, XB], [1, REC0]]),
                ht[:, :])

        # ---------------- edge-phase machinery -------------------------------
        gpool = ctx.enter_context(tc.tile_pool(name="gpool", bufs=gbufs))
        ppool = ctx.enter_context(tc.tile_pool(name="ppool", bufs=3))
        small = ctx.enter_context(tc.tile_pool(name="small", bufs=3))
        def edge_group(gr, table, rec, a_sb, dst, dst_is_x1):
            b0, nb, klo, khi = gr["b0"], gr["nb"], gr["klo"], gr["khi"]
            used = nb * (klo + khi)
            nlo = nb * klo
            ac_col = 128 if rec == REC0 else 2
            nf = HID if rec == REC0 else 1

            # Gathers split into 4 column-chunks, one per SWDGE queue: each
            # queue is generated by its own Q7 core pair (ucode dispatches by
            # cpu_id/2 == queue_num), so chunks generate in parallel.
            g = gpool.tile([128, used, rec], F16, tag="g", name="g")

            # Balanced 4-way chunking: each queue (= Q7 core pair) gets
            # ~1/4 of the group's columns, spanning the lo/hi boundary when
            # needed; 4-5 gathers per group keeps the DMASW lane-reuse
            # distance at ~2 groups.
            def emit(q, is_lo, s, n):
                off = gr["off_lo" if is_lo else "off_hi"]
                c0 = 0 if is_lo else nlo
                tab_ap = (table[0:HALFT, :] if is_lo
                          else table[HALFT:2 * HALFT, :])
                nc.gpsimd.dma_gather(
                    g[:, c0 + s:c0 + s + n, :], tab_ap,
                    idx_sb[:, off + 8 * s:off + 8 * (s + n)],
                    num_idxs=128 * n, num_idxs_reg=128 * n,
                    elem_size=rec, single_packet=False, queue_num=q)

            # Exactly 4 gathers per group in strict queue order 0..3 (Tile
            # locks DMASW lanes to queues by emission rotation). The lo/hi
            # boundary picks how many queues serve each half.
            nhi = nb * khi
            parts = []
            if nlo == 0 or nhi == 0:
                side, cols = (nlo > 0), max(nlo, nhi)
                splits = 4
                base = 0
                for q in range(4):
                    n = (cols + splits - 1 - q) // splits if False else None
                parts = []
                st = 0
                for q in range(4):
                    n = cols // 4 + (1 if q < cols % 4 else 0)
                    parts.append((side, st, n))
                    st += n
            else:
                a = max(1, min(3, round(4 * nlo / used)))
                st = 0
                for q in range(a):
                    n = nlo // a + (1 if q < nlo % a else 0)
                    parts.append((True, st, n))
                    st += n
                st = 0
                for q in range(4 - a):
                    n = nhi // (4 - a) + (1 if q < nhi % (4 - a) else 0)
                    parts.append((False, st, n))
                    st += n
            for q, (is_lo, st, n) in enumerate(parts):
                if n > 0:
                    emit(q, is_lo, st, n)

            # tt/pt are laid out [p, n, KF=(klo+khi), ...] so each batch's
            # lo+hi slots are contiguous: s and acc reduce in ONE op each.
            KF = klo + khi
            tt = small.tile([128, nb, KF, HEADS], F32, tag="tt", name="tt")
            a_r = a_sb[:, b0:b0 + nb, :]
            pt = ppool.tile([128, nb * KF * HEADS * nf], F16, tag="p",
                            name="pt")
            w = small.tile([128, nb, KF, HEADS], F16, tag="w", name="w")
            for (c0, k0, kk) in ((0, 0, klo), (nlo, klo, khi)):
                if kk == 0:
                    continue
                # t = a_c + a_r
                nc.vector.tensor_tensor(
                    tt[:, :, k0:k0 + kk, :],
                    bass.AP(g.tensor, g.offset + c0 * rec + ac_col,
                            [g.ap[0], [rec * kk, nb], [rec, kk], [1, HEADS]]),
                    a_r.unsqueeze(2).broadcast_to([128, nb, kk, HEADS]),
                    op=mybir.AluOpType.add)
            # w = exp(max(t, 0.2t))  (fp16)
            lr = small.tile([128, nb, KF, HEADS], F32, tag="lr", name="lr")
            nc.vector.scalar_tensor_tensor(
                lr[:, :, :, :], tt[:, :, :, :], 0.2, tt[:, :, :, :],
                op0=mybir.AluOpType.mult, op1=mybir.AluOpType.max)
            nc.scalar.activation(w[:, :, :, :], lr[:, :, :, :],
                                 mybir.ActivationFunctionType.Exp)
            for (c0, k0, kk) in ((0, 0, klo), (nlo, klo, khi)):
                if kk == 0:
                    continue
                nc.vector.tensor_tensor(
                    bass.AP(pt.tensor, pt.offset + k0 * HEADS * nf,
                            [pt.ap[0], [KF * HEADS * nf, nb],
                             [HEADS * nf, kk], [nf, HEADS], [1, nf]]),
                    bass.AP(g.tensor, g.offset + c0 * rec,
                            [g.ap[0], [rec * kk, nb], [rec, kk], [nf, HEADS],
                             [1, nf]]),
                    w[:, :, k0:k0 + kk, :]
                     .unsqueeze(4).broadcast_to([128, nb, kk, HEADS, nf]),
                    op=mybir.AluOpType.mult)
            sacc = small.tile([128, nb, HEADS], F32, tag="sa", name="sacc")
            nc.vector.reduce_sum(
                sacc[:, :, :],
                w[:, :, :, :].rearrange("p n k h -> p n h k"),
                axis=mybir.AxisListType.X)
            acc = small.tile([128, nb, HEADS, nf], F32, tag="aa", name="acc")
            nc.vector.reduce_sum(
                acc[:, :, :, :],
                pt[:, :].rearrange("p (n k h c) -> p n h c k", n=nb, k=KF,
                                   h=HEADS),
                axis=mybir.AxisListType.X)

            # combine heads: dst = act(0.5*(acc0/s0 + acc1/s1))
            nc.vector.tensor_scalar_max(sacc[:, :, :], sacc[:, :, :], 1e-30)
            rs = small.tile([128, nb, HEADS], F32, tag="rs", name="rs")
            nc.vector.reciprocal(rs[:, :, :], sacc[:, :, :])
            tmp = small.tile([128, nb, nf], F32, tag="tm", name="tmp")
            nc.vector.tensor_tensor(
                tmp[:, :, :], acc[:, :, 1, :],
                rs[:, :, 1:2].broadcast_to([128, nb, nf]),
                op=mybir.AluOpType.mult)
            xs = small.tile([128, nb, nf], F32, tag="xs", name="xs")
            nc.vector.tensor_tensor(
                xs[:, :, :], acc[:, :, 0, :],
                rs[:, :, 0:1].broadcast_to([128, nb, nf]),
                op=mybir.AluOpType.mult)
            nc.vector.tensor_add(xs[:, :, :], xs[:, :, :], tmp[:, :, :])
            if dst_is_x1:
                nc.scalar.activation(dst[:, b0:b0 + nb, :], xs[:, :, :],
                                     mybir.ActivationFunctionType.Relu,
                                     scale=0.5)
            else:
                nc.scalar.activation(dst[:, b0:b0 + nb], xs[:, :, 0],
                                     mybir.ActivationFunctionType.Copy,
                                     scale=0.5)

        def edge_phase(groups, table, rec, a_sb, dst, dst_is_x1):
            for gr in groups:
                edge_group(gr, table, rec, a_sb, dst, dst_is_x1)

        # ---------------- phase 2: layer-0 edge phase ------------------------
        edge_phase(sched.groups0, htab, REC0, a0_sb, x1_all, True)

        # ---------------- layer-1 node matmuls -------------------------------
        for b in range(B):
            ps_t1 = psum.tile([64, 128], F16, tag="tp")
            nc.tensor.transpose(ps_t1[:, :], x1_all[:, b, :], ident[:, :])
            xt1 = stage.tile([64, 128], F16, tag="xt1")
            nc.vector.tensor_copy(xt1[:, :], ps_t1[:, :])
            ps_m1 = psum.tile([128, REC1], F32, tag="m1")
            nc.tensor.matmul(ps_m1[:, :], xt1[:, :], wcat1[:, :],
                             start=True, stop=True)
            t1b = stage.tile([128, REC1], F16, tag="t1b")
            nc.scalar.activation(t1b[:, :], ps_m1[:, :],
                                 mybir.ActivationFunctionType.Copy)
            nc.vector.tensor_copy(a1_sb[:, b, :], ps_m1[:, 4:6])
            if b == B - 1:
                nc.vector.scalar_tensor_tensor(
                    t1b[:, 2:4],
                    ident[:, DPART:DPART + 1].broadcast_to([128, HEADS]),
                    NEGC, t1b[:, 2:4],
                    op0=mybir.AluOpType.mult, op1=mybir.AluOpType.add)
            nc.sync.dma_start(t1slice[b * 128:(b + 1) * 128, :], t1b[:, :])

        nc.gpsimd.collective_compute(
            "AllGather", mybir.AluOpType.bypass, replica_groups=rg,
            ins=[t1slice[:, :]], outs=[t1tab[:, :]])

        # ---------------- phase 3: layer-1 edge phase ------------------------
        edge_phase(sched.groups1, t1tab, REC1, a1_sb, out_sb, False)

        nc.sync.dma_start(out_d[:, :], out_sb[:, :])

    nc.compile()
    return nc


def assemble_output(sched, core_outs, unperm):
    """core_outs: list of [128, B] arrays -> full [N, 1] output."""
    full = np.concatenate(
        [co.T.reshape(-1)[:sched.S] for co in core_outs])   # rank order
    res = np.empty((sched.N, 1), np.float32)
    res[unperm] = full[:, None]
    return res


# ----------------------------------------------------------------------------
# Harness entry point
# ----------------------------------------------------------------------------

_CACHE = {}


def kernel(x, edge_index, W0, attn0, W1, attn1):
    """Full-input / full-output GAT forward on 8 TRN2 cores."""
    from concourse.bass_interp import get_hw_module
    from concourse.bass_utils import run_bass_kernel_spmd

    NC = 8
    x = np.asarray(x, np.float32)
    edge_index = np.asarray(edge_index)
    sched, per_core, unperm = build_host_data(
        x, edge_index, np.asarray(W0, np.float32), np.asarray(attn0, np.float32),
        np.asarray(W1, np.float32), np.asarray(attn1, np.float32), NC=NC)

    key = (sched.N, sched.W, tuple(sched.KL), tuple(sched.KH))
    nc = _CACHE.get(key)
    if nc is None:
        nc = build_kernel(sched)
        nc.m = get_hw_module(nc.m)
        _CACHE[key] = nc

    res = run_bass_kernel_spmd(nc, per_core, core_ids=list(range(NC)),
                               trace=False)
    outs = [res.results[r]["out"] for r in range(NC)]
    return assemble_output(sched, outs, unperm)
